# revision 45
# baseline (speedup 1.0000x reference)
"""Epipolar attention kernel for Trainium2 (8 NeuronCores, batch-parallel).

Host does the O(B) 3x3 geometry (SVD etc.) in float32 numpy, mirroring the
reference op-for-op; the device does all O(N^2) / O(N^2*C) work:
  d5[i,j]   = |5*(A_j*x_i + B_j*y_i + C_j)|        (PE, exact triple-bf16 split)
  e[i,j]    = exp(d5 - rowmax(d5)), r_i = rowsum   (ACT, fused accum)
  E2[i,j]   = exp(-e/r)                            (ACT, per-partition scale)
  qT        = (E2^T - 1) in fp8, S = colsum(E2)    (PE transpose + DVE fused)
  out       = T + qT @ (256/S * fsrcT) / 256       (fp8 DoubleRow matmuls;
              T[c] = sum_j fsrcT[j,c]/S_j seeds the PSUM accumulation)
The double softmax identity: softmax_i(1 - p) == softmax_i(-p) == E2/colsum.
The E2 = 1 + q split keeps fp8 quantization error confined to the small
attention-deviation term (q weighted ~2.5% of output norm), so e4m3
matmuls cost ~1e-3 relative error instead of ~4e-2.
"""

import numpy as np
import ml_dtypes

import concourse.bass as bass
import concourse.bacc as bacc
import concourse.tile as tile
from concourse import mybir
from concourse.bass_utils import run_bass_kernel_spmd

B, C, H, W = 8, 1152, 32, 32
N = H * W           # 1024
P = 128
NT = N // P         # 8
F32 = mybir.dt.float32
F16 = mybir.dt.float16
BF16 = mybir.dt.bfloat16
FP8 = mybir.dt.float8e4
BFNP = ml_dtypes.bfloat16
KAPPA = 256.0       # fp8 scaling of fs8 = KAPPA/S * fsrcT; host divides out

TRACE = False
LAST_RESULTS = None


# ----------------------------------------------------------------- device ---

def _build_nc():
    nc = bacc.Bacc()
    fsrcT = nc.dram_tensor("fsrcT", (N, C), F16, kind="ExternalInput")
    xyabc = nc.dram_tensor("xyabc", (9, 2 * N), BF16, kind="ExternalInput")
    identD = nc.dram_tensor("ident", (P, P), F16, kind="ExternalInput")
    out = nc.dram_tensor("out", (N, C), F16, kind="ExternalOutput")

    AF = mybir.ActivationFunctionType
    AO = mybir.AluOpType
    PM = mybir.MatmulPerfMode
    I32 = mybir.dt.int32

    with tile.TileContext(nc) as tc:
        with (
            tc.tile_pool(name="consts", bufs=1) as consts,
            tc.tile_pool(name="persist", bufs=1) as persist,
            tc.tile_pool(name="work", bufs=4) as work,
            tc.tile_pool(name="stats", bufs=8) as stats,
        ):
            xyabc_sb = consts.tile([9, 2 * N], BF16, tag="xyabc")
            nc.sync.dma_start(out=xyabc_sb, in_=xyabc[:, :])
            ident = consts.tile([P, P], F16, tag="ident")
            nc.scalar.dma_start(out=ident, in_=identD[:, :])
            ones = consts.tile([P, P], F16, tag="ones")
            nc.vector.memset(ones, 1.0)

            fs_sb = persist.tile([P, NT, C], F16, tag="fs")
            for h in range(4):
                nc.sync.dma_start(
                    out=fs_sb[:, 2 * h:2 * h + 2, :],
                    in_=fsrcT[2 * h * P:(2 * h + 2) * P, :].rearrange(
                        "(j p) c -> p j c", p=P))
            e2_sb = persist.tile([P, NT, N], F16, tag="e2")
            at8 = persist.tile([P, NT, N], FP8, tag="at8")
            fs8 = persist.tile([P, NT, C], FP8, tag="fs8")
            iSbc = persist.tile([P, NT, P], F16, tag="iSbc")
            Tsb = persist.tile([P, C], BF16, tag="Tsb")

            # Phase A: rows i on partitions, j on free dim.  E2 issue is
            # software-pipelined one iteration behind so ACT never stalls on
            # the DVE reciprocal of the same tile.
            e2_pend = []
            with tc.tile_pool(name="psA", bufs=2, space="PSUM") as psA:
                for it in range(NT):
                    d_ps = psA.tile([P, N], F32)
                    for h in range(2):
                        nc.tensor.matmul(
                            d_ps[:, h * 512:(h + 1) * 512],
                            lhsT=xyabc_sb[:, it * P:(it + 1) * P],
                            rhs=xyabc_sb[:, N + h * 512:N + (h + 1) * 512],
                            start=True, stop=True,
                        )
                    dabs = work.tile([P, N], F32, tag="dabs")
                    nc.vector.tensor_scalar(
                        out=dabs.bitcast(I32), in0=d_ps.bitcast(I32),
                        scalar1=0x7FFFFFFF, scalar2=None, op0=AO.bitwise_and,
                    )
                    nmx = stats.tile([P, 1], F32, tag="nmx")
                    nc.vector.tensor_reduce(
                        out=nmx, in_=dabs, axis=mybir.AxisListType.X,
                        op=AO.max, negate=True,
                    )
                    e_t = work.tile([P, N], F16, tag="e")
                    r = stats.tile([P, 1], F32, tag="r")
                    nc.scalar.activation(
                        out=e_t, in_=dabs, func=AF.Exp, bias=nmx, scale=1.0,
                        accum_out=r
                    )
                    negr = stats.tile([P, 1], F32, tag="negr")
                    nc.vector.tensor_scalar_mul(negr, r, -1.0)
                    ninvr = stats.tile([P, 1], F32, tag="ninvr")
                    nc.vector.reciprocal(ninvr, negr)     # -1/r
                    e2_pend.append((it, e_t, ninvr))
                    if len(e2_pend) == 2:
                        pit, pe, pninvr = e2_pend.pop(0)
                        nc.scalar.activation(
                            out=e2_sb[:, pit, :], in_=pe, func=AF.Exp,
                            bias=0.0, scale=pninvr
                        )
                for pit, pe, pninvr in e2_pend:
                    nc.scalar.activation(
                        out=e2_sb[:, pit, :], in_=pe, func=AF.Exp, bias=0.0,
                        scale=pninvr
                    )

            # Phase B: PE-transpose E2 stripes; the PSUM->SBUF copy also
            # subtracts 1 (q = E2T - 1, stored fp8) and accumulates the
            # column sum; invS256 = KAPPA/S feeds both fs8 and the T-chain.
            # The T-chain matmuls (T = sum_j fsrcT[j,:]/S_j, broadcast over
            # partitions via a 256/S stationary) interleave with transposes.
            with tc.tile_pool(name="psB", bufs=2, space="PSUM") as psT, \
                 tc.tile_pool(name="psTch", bufs=1, space="PSUM") as psTch:
                Tps = [psTch.tile([P, cw], F32, tag=f"Tch{ci}",
                                  name=f"Tch{ci}")
                       for ci, cw in enumerate((512, 512, 128))]
                for u in range(NT):
                    tp = psT.tile([P, N], F16)
                    for it in range(NT):
                        nc.tensor.transpose(
                            tp[:, it * P:(it + 1) * P],
                            e2_sb[:, it, u * P:(u + 1) * P],
                            ident,
                        )
                    Ssub = stats.tile([P, 1], F32, tag="Ssub")
                    nc.vector.tensor_scalar(
                        out=at8[:, u, :], in0=tp, scalar1=-1.0, scalar2=0.0,
                        op0=AO.add, op1=AO.add, accum_out=Ssub,
                    )
                    Sdiv = stats.tile([P, 1], F32, tag="Sdiv")
                    nc.vector.tensor_scalar(
                        out=Sdiv, in0=Ssub, scalar1=float(N),
                        scalar2=1.0 / KAPPA, op0=AO.add, op1=AO.mult,
                    )
                    iS = stats.tile([P, 1], F32, tag="iS")
                    nc.vector.reciprocal(iS, Sdiv)        # KAPPA / S
                    nc.vector.tensor_scalar_mul(iSbc[:, u, :], ones, iS)
                    if u % 2 == 0:
                        nc.vector.tensor_scalar_mul(
                            fs8[:, u, :], fs_sb[:, u, :], iS)
                    else:
                        nc.scalar.activation(
                            out=fs8[:, u, :], in_=fs_sb[:, u, :],
                            func=AF.Copy, scale=iS)
                    for ci, (c0, cw) in enumerate(
                            ((0, 512), (512, 512), (1024, 128))):
                        nc.tensor.matmul(
                            Tps[ci][:, :],
                            lhsT=iSbc[:, u, :],
                            rhs=fs_sb[:, u, c0:c0 + cw],
                            start=(u == 0), stop=(u == NT - 1),
                        )
                # Chunk 1 is PE-seeded (ones @ Tsb/128 reproduces KAPPA*T);
                # chunks 0/2 get KAPPA*T added during the DVE drain instead.
                for ci, (c0, cw) in enumerate(
                        ((0, 512), (512, 512), (1024, 128))):
                    nc.scalar.activation(
                        out=Tsb[:, c0:c0 + cw], in_=Tps[ci], func=AF.Copy,
                        scale=(1.0 / P) if ci == 1 else 1.0)

            # Phase C: PSUM starts from the rank-1 term KAPPA*T (ones-seed),
            # then fp8 DoubleRow matmuls accumulate qT @ fs8 (two j-tiles of
            # contraction per pass).  Host divides the f16 output by KAPPA.
            CCH = ((0, 512), (512, 512), (1024, 128))
            with tc.tile_pool(name="psC", bufs=2, space="PSUM") as psC:
                for it in range(NT):
                    ocs = [psC.tile([P, cw], F32, tag=f"oc{ci}",
                                    name=f"oc_{it}_{ci}")
                           for ci, (c0, cw) in enumerate(CCH)]
                    nc.tensor.matmul(
                        ocs[1][:, :],
                        lhsT=ones,
                        rhs=Tsb[:, 512:1024],
                        start=True, stop=False,
                    )
                    for hp in range(NT // 2):
                        for ck, (c0, cw) in enumerate(CCH):
                            nc.tensor.matmul(
                                ocs[ck][:, :],
                                lhsT=at8[:, 2 * hp:2 * hp + 2,
                                         it * P:(it + 1) * P],
                                rhs=fs8[:, 2 * hp:2 * hp + 2, c0:c0 + cw],
                                start=(hp == 0 and ck != 1),
                                stop=(hp == NT // 2 - 1),
                                perf_mode=PM.DoubleRow,
                            )
                    osb = work.tile([P, C], F16, tag="osb")
                    for ck, (c0, cw) in enumerate(CCH):
                        if ck == 1:
                            nc.scalar.copy(osb[:, c0:c0 + cw], ocs[ck])
                        else:
                            nc.vector.tensor_add(
                                osb[:, c0:c0 + cw], ocs[ck],
                                Tsb[:, c0:c0 + cw])
                    nc.sync.dma_start(
                        out=out[it * P:(it + 1) * P, :], in_=osb[:, :])
    nc.compile()
    return nc


_NC = None


def _get_nc():
    global _NC
    if _NC is None:
        _NC = _build_nc()
    return _NC


# ------------------------------------------------------------------- host ---

def _skew(t):
    z = np.zeros_like(t[:, 0])
    return np.stack([
        np.stack([z, -t[:, 2], t[:, 1]], -1),
        np.stack([t[:, 2], z, -t[:, 0]], -1),
        np.stack([-t[:, 1], t[:, 0], z], -1),
    ], 1)


def _fundamental(K1, K2, R, t):
    E = _skew(t) @ R
    U, S, Vt = np.linalg.svd(E)
    S = S.copy()
    S[:, 2] = 0.0
    E = U @ (S[:, :, None] * Vt)
    return np.linalg.inv(np.swapaxes(K2, 1, 2)) @ E @ np.linalg.inv(K1)


def _split3(v):
    """Exact-ish triple bf16 split: v ~= hi + mid + lo (24 mantissa bits)."""
    v = v.astype(np.float32)
    hi = v.astype(BFNP)
    r1 = v - hi.astype(np.float32)
    mid = r1.astype(BFNP)
    r2 = r1 - mid.astype(np.float32)
    lo = r2.astype(BFNP)
    return hi, mid, lo


def _host_prep(f_src, K1, K2, R, t):
    ix, iy = np.meshgrid(np.arange(H, dtype=np.float32),
                         np.arange(W, dtype=np.float32), indexing="ij")
    comb = np.stack([ix.ravel(), iy.ravel(), np.ones(N, np.float32)], 0)  # (3,N)

    F = _fundamental(K1, K2, R, t)                    # (B,3,3)
    lines = (F @ comb).astype(np.float32)             # (B,3,N)
    lines = lines / lines[:, 2:3, :]
    y0 = -lines[:, 2, :] / lines[:, 1, :]
    y1 = -(lines[:, 2, :] + lines[:, 0, :] * np.float32(W)) / lines[:, 1, :]
    dy = y0 - y1
    L = np.sqrt(np.float32(W * W) + dy * dy)
    A5 = np.float32(5.0) * (dy / L)
    B5 = np.float32(5.0) * (np.float32(W) / L)
    C5 = np.float32(-5.0) * (np.float32(W) * y0 / L)

    Ah, Am, Al = _split3(A5)
    Bh, Bm, Bl = _split3(B5)
    Ch, Cm, Cl = _split3(C5)
    abc9 = np.stack([Ah, Bh, Ch, Am, Bm, Cm, Al, Bl, Cl], axis=1)  # (B,9,N) bf16
    xy9 = np.tile(comb, (3, 1)).astype(BFNP)                        # (9,N) exact

    fsT = np.ascontiguousarray(
        f_src.reshape(B, C, N).transpose(0, 2, 1)).astype(np.float16)  # (B,N,C)
    return abc9, xy9, fsT


def kernel(f_tar=None, f_src=None, K1=None, K2=None, R=None, t=None):
    global LAST_RESULTS
    f_src = np.asarray(f_src, np.float32)
    K1 = np.asarray(K1, np.float32)
    K2 = np.asarray(K2, np.float32)
    R = np.asarray(R, np.float32)
    t = np.asarray(t, np.float32)

    abc9, xy9, fsT = _host_prep(f_src, K1, K2, R, t)
    ident = np.eye(P, dtype=np.float16)
    in_maps = [
        {"fsrcT": fsT[b],
         "xyabc": np.ascontiguousarray(
             np.concatenate([xy9, abc9[b]], 1)),
         "ident": ident}
        for b in range(B)
    ]
    res = run_bass_kernel_spmd(_get_nc(), in_maps, list(range(B)), trace=TRACE)
    LAST_RESULTS = res
    outs = np.stack([res.results[b]["out"] for b in range(B)], 0)  # (B,N,C)
    return (outs.astype(np.float32) / np.float32(KAPPA)).reshape(B, C, H, W)


# revision 47
# speedup vs baseline: 1.1649x; 1.1649x over previous
"""Epipolar attention kernel for Trainium2 (8 NeuronCores, batch-parallel).

Host does the O(B) 3x3 geometry (SVD etc.) in float32 numpy, mirroring the
reference op-for-op; the device does all O(N^2) / O(N^2*C) work:
  d5[i,j]   = |5*(A_j*x_i + B_j*y_i + C_j)|        (PE, exact triple-bf16 split)
  e[i,j]    = exp(d5 - rowmax(d5)), r_i = rowsum   (ACT, fused accum)
  E2[i,j]   = exp(-e/r)                            (ACT, per-partition scale)
  qT        = (E2^T - 1) in fp8, S = colsum(E2)    (PE transpose + DVE fused)
  out       = T + qT @ (256/S * fsrcT) / 256       (fp8 DoubleRow matmuls;
              T[c] = sum_j fsrcT[j,c]/S_j seeds the PSUM accumulation)
The double softmax identity: softmax_i(1 - p) == softmax_i(-p) == E2/colsum.
The E2 = 1 + q split keeps fp8 quantization error confined to the small
attention-deviation term (q weighted ~2.5% of output norm), so e4m3
matmuls cost ~1e-3 relative error instead of ~4e-2.
"""

import numpy as np
import ml_dtypes

import concourse.bass as bass
import concourse.bacc as bacc
import concourse.tile as tile
from concourse import mybir
from concourse.bass_utils import run_bass_kernel_spmd

B, C, H, W = 8, 1152, 32, 32
N = H * W           # 1024
P = 128
NT = N // P         # 8
F32 = mybir.dt.float32
F16 = mybir.dt.float16
BF16 = mybir.dt.bfloat16
FP8 = mybir.dt.float8e4
BFNP = ml_dtypes.bfloat16
KAPPA = 256.0       # fp8 scaling of fs8 = KAPPA/S * fsrcT; host divides out

TRACE = False
LAST_RESULTS = None


# ----------------------------------------------------------------- device ---

def _build_nc():
    nc = bacc.Bacc()
    fsrcT = nc.dram_tensor("fsrcT", (N, C), F16, kind="ExternalInput")
    xyabc = nc.dram_tensor("xyabc", (9, 2 * N), BF16, kind="ExternalInput")
    identD = nc.dram_tensor("ident", (P, P), F16, kind="ExternalInput")
    out = nc.dram_tensor("out", (N, C), F16, kind="ExternalOutput")

    AF = mybir.ActivationFunctionType
    AO = mybir.AluOpType
    PM = mybir.MatmulPerfMode
    I32 = mybir.dt.int32

    with tile.TileContext(nc) as tc:
        with (
            tc.tile_pool(name="consts", bufs=1) as consts,
            tc.tile_pool(name="persist", bufs=1) as persist,
            tc.tile_pool(name="work", bufs=4) as work,
            tc.tile_pool(name="stats", bufs=8) as stats,
        ):
            xyabc_sb = consts.tile([9, 2 * N], BF16, tag="xyabc")
            nc.sync.dma_start(out=xyabc_sb, in_=xyabc[:, :])
            ident = consts.tile([P, P], F16, tag="ident")
            nc.scalar.dma_start(out=ident, in_=identD[:, :])
            ones = consts.tile([P, P], F16, tag="ones")
            nc.vector.memset(ones, 1.0)

            fs_sb = persist.tile([P, NT, C], F16, tag="fs")
            for h in range(4):
                nc.sync.dma_start(
                    out=fs_sb[:, 2 * h:2 * h + 2, :],
                    in_=fsrcT[2 * h * P:(2 * h + 2) * P, :].rearrange(
                        "(j p) c -> p j c", p=P))
            e2_sb = persist.tile([P, NT, N], F16, tag="e2")
            at8 = persist.tile([P, NT, N], FP8, tag="at8")
            fs8 = persist.tile([P, NT, C], FP8, tag="fs8")
            iSbc = persist.tile([P, NT, P], F16, tag="iSbc")
            Tsb = persist.tile([P, C], BF16, tag="Tsb")

            # Phase A: rows i on partitions, j on free dim.  E2 issue is
            # software-pipelined one iteration behind so ACT never stalls on
            # the DVE reciprocal of the same tile.
            e2_pend = []
            with tc.tile_pool(name="psA", bufs=2, space="PSUM") as psA:
                for it in range(NT):
                    d_ps = psA.tile([P, N], F32)
                    for h in range(2):
                        nc.tensor.matmul(
                            d_ps[:, h * 512:(h + 1) * 512],
                            lhsT=xyabc_sb[:, it * P:(it + 1) * P],
                            rhs=xyabc_sb[:, N + h * 512:N + (h + 1) * 512],
                            start=True, stop=True,
                        )
                    dabs = work.tile([P, N], F32, tag="dabs")
                    nc.vector.tensor_scalar(
                        out=dabs.bitcast(I32), in0=d_ps.bitcast(I32),
                        scalar1=0x7FFFFFFF, scalar2=None, op0=AO.bitwise_and,
                    )
                    nmx = stats.tile([P, 1], F32, tag="nmx")
                    nc.vector.tensor_reduce(
                        out=nmx, in_=dabs, axis=mybir.AxisListType.X,
                        op=AO.max, negate=True,
                    )
                    e_t = work.tile([P, N], F16, tag="e")
                    r = stats.tile([P, 1], F32, tag="r")
                    nc.scalar.activation(
                        out=e_t, in_=dabs, func=AF.Exp, bias=nmx, scale=1.0,
                        accum_out=r
                    )
                    negr = stats.tile([P, 1], F32, tag="negr")
                    nc.vector.tensor_scalar_mul(negr, r, -1.0)
                    ninvr = stats.tile([P, 1], F32, tag="ninvr")
                    nc.vector.reciprocal(ninvr, negr)     # -1/r
                    e2_pend.append((it, e_t, ninvr))
                    if len(e2_pend) == 2:
                        pit, pe, pninvr = e2_pend.pop(0)
                        nc.scalar.activation(
                            out=e2_sb[:, pit, :], in_=pe, func=AF.Exp,
                            bias=0.0, scale=pninvr
                        )
                for pit, pe, pninvr in e2_pend:
                    nc.scalar.activation(
                        out=e2_sb[:, pit, :], in_=pe, func=AF.Exp, bias=0.0,
                        scale=pninvr
                    )

            # Phase B: PE-transpose E2 stripes; the PSUM->SBUF copy also
            # subtracts 1 (q = E2T - 1, stored fp8) and accumulates the
            # column sum; invS256 = KAPPA/S feeds both fs8 and the T-chain.
            # The T-chain matmuls (T = sum_j fsrcT[j,:]/S_j, broadcast over
            # partitions via a 256/S stationary) interleave with transposes.
            with tc.tile_pool(name="psB", bufs=2, space="PSUM") as psT, \
                 tc.tile_pool(name="psTch", bufs=1, space="PSUM") as psTch:
                Tps = [psTch.tile([P, cw], F32, tag=f"Tch{ci}",
                                  name=f"Tch{ci}")
                       for ci, cw in enumerate((512, 512, 128))]
                for u in range(NT):
                    tp = psT.tile([P, N], F16)
                    for it in range(NT):
                        nc.tensor.transpose(
                            tp[:, it * P:(it + 1) * P],
                            e2_sb[:, it, u * P:(u + 1) * P],
                            ident,
                        )
                    Ssub = stats.tile([P, 1], F32, tag="Ssub")
                    nc.vector.tensor_scalar(
                        out=at8[:, u, :], in0=tp, scalar1=-1.0, scalar2=0.0,
                        op0=AO.add, op1=AO.add, accum_out=Ssub,
                    )
                    Sdiv = stats.tile([P, 1], F32, tag="Sdiv")
                    nc.vector.tensor_scalar(
                        out=Sdiv, in0=Ssub, scalar1=float(N),
                        scalar2=1.0 / KAPPA, op0=AO.add, op1=AO.mult,
                    )
                    iS = stats.tile([P, 1], F32, tag="iS")
                    nc.vector.reciprocal(iS, Sdiv)        # KAPPA / S
                    nc.vector.tensor_scalar_mul(iSbc[:, u, :], ones, iS)
                    if u % 2 == 0:
                        nc.vector.tensor_scalar_mul(
                            fs8[:, u, :], fs_sb[:, u, :], iS)
                    else:
                        nc.scalar.activation(
                            out=fs8[:, u, :], in_=fs_sb[:, u, :],
                            func=AF.Copy, scale=iS)
                    for ci, (c0, cw) in enumerate(
                            ((0, 512), (512, 512), (1024, 128))):
                        nc.tensor.matmul(
                            Tps[ci][:, :],
                            lhsT=iSbc[:, u, :],
                            rhs=fs_sb[:, u, c0:c0 + cw],
                            start=(u == 0), stop=(u == NT - 1),
                        )
                # Tsb = KAPPA*T/128: the ones-seed matmul reproduces KAPPA*T
                for ci, (c0, cw) in enumerate(
                        ((0, 512), (512, 512), (1024, 128))):
                    nc.scalar.activation(
                        out=Tsb[:, c0:c0 + cw], in_=Tps[ci], func=AF.Copy,
                        scale=1.0 / P)

            # Phase C: PSUM starts from the rank-1 term KAPPA*T (ones-seed),
            # then fp8 DoubleRow matmuls accumulate qT @ fs8 (two j-tiles of
            # contraction per pass).  Host divides the f16 output by KAPPA.
            CCH = ((0, 512), (512, 512), (1024, 128))
            with tc.tile_pool(name="psC", bufs=2, space="PSUM") as psC:
                for it in range(NT):
                    ocs = [psC.tile([P, cw], F32, tag=f"oc{ci}",
                                    name=f"oc_{it}_{ci}")
                           for ci, (c0, cw) in enumerate(CCH)]
                    for ck, (c0, cw) in enumerate(CCH):
                        nc.tensor.matmul(
                            ocs[ck][:, :],
                            lhsT=ones,
                            rhs=Tsb[:, c0:c0 + cw],
                            start=True, stop=False,
                        )
                    for hp in range(NT // 2):
                        for ck, (c0, cw) in enumerate(CCH):
                            nc.tensor.matmul(
                                ocs[ck][:, :],
                                lhsT=at8[:, 2 * hp:2 * hp + 2,
                                         it * P:(it + 1) * P],
                                rhs=fs8[:, 2 * hp:2 * hp + 2, c0:c0 + cw],
                                start=False,
                                stop=(hp == NT // 2 - 1),
                                perf_mode=PM.DoubleRow,
                            )
                    osb = work.tile([P, C], F16, tag="osb")
                    for ck, (c0, cw) in enumerate(CCH):
                        if ck == 1:
                            nc.vector.tensor_copy(osb[:, c0:c0 + cw], ocs[ck])
                        else:
                            nc.scalar.copy(osb[:, c0:c0 + cw], ocs[ck])
                    nc.sync.dma_start(
                        out=out[it * P:(it + 1) * P, :], in_=osb[:, :])
    nc.compile()
    return nc


_NC = None


def _get_nc():
    global _NC
    if _NC is None:
        _NC = _build_nc()
    return _NC


# ------------------------------------------------------------------- host ---

def _skew(t):
    z = np.zeros_like(t[:, 0])
    return np.stack([
        np.stack([z, -t[:, 2], t[:, 1]], -1),
        np.stack([t[:, 2], z, -t[:, 0]], -1),
        np.stack([-t[:, 1], t[:, 0], z], -1),
    ], 1)


def _fundamental(K1, K2, R, t):
    E = _skew(t) @ R
    U, S, Vt = np.linalg.svd(E)
    S = S.copy()
    S[:, 2] = 0.0
    E = U @ (S[:, :, None] * Vt)
    return np.linalg.inv(np.swapaxes(K2, 1, 2)) @ E @ np.linalg.inv(K1)


def _split3(v):
    """Exact-ish triple bf16 split: v ~= hi + mid + lo (24 mantissa bits)."""
    v = v.astype(np.float32)
    hi = v.astype(BFNP)
    r1 = v - hi.astype(np.float32)
    mid = r1.astype(BFNP)
    r2 = r1 - mid.astype(np.float32)
    lo = r2.astype(BFNP)
    return hi, mid, lo


def _host_prep(f_src, K1, K2, R, t):
    ix, iy = np.meshgrid(np.arange(H, dtype=np.float32),
                         np.arange(W, dtype=np.float32), indexing="ij")
    comb = np.stack([ix.ravel(), iy.ravel(), np.ones(N, np.float32)], 0)  # (3,N)

    F = _fundamental(K1, K2, R, t)                    # (B,3,3)
    lines = (F @ comb).astype(np.float32)             # (B,3,N)
    lines = lines / lines[:, 2:3, :]
    y0 = -lines[:, 2, :] / lines[:, 1, :]
    y1 = -(lines[:, 2, :] + lines[:, 0, :] * np.float32(W)) / lines[:, 1, :]
    dy = y0 - y1
    L = np.sqrt(np.float32(W * W) + dy * dy)
    A5 = np.float32(5.0) * (dy / L)
    B5 = np.float32(5.0) * (np.float32(W) / L)
    C5 = np.float32(-5.0) * (np.float32(W) * y0 / L)

    Ah, Am, Al = _split3(A5)
    Bh, Bm, Bl = _split3(B5)
    Ch, Cm, Cl = _split3(C5)
    abc9 = np.stack([Ah, Bh, Ch, Am, Bm, Cm, Al, Bl, Cl], axis=1)  # (B,9,N) bf16
    xy9 = np.tile(comb, (3, 1)).astype(BFNP)                        # (9,N) exact

    fsT = np.ascontiguousarray(
        f_src.reshape(B, C, N).transpose(0, 2, 1)).astype(np.float16)  # (B,N,C)
    return abc9, xy9, fsT


def kernel(f_tar=None, f_src=None, K1=None, K2=None, R=None, t=None):
    global LAST_RESULTS
    f_src = np.asarray(f_src, np.float32)
    K1 = np.asarray(K1, np.float32)
    K2 = np.asarray(K2, np.float32)
    R = np.asarray(R, np.float32)
    t = np.asarray(t, np.float32)

    abc9, xy9, fsT = _host_prep(f_src, K1, K2, R, t)
    ident = np.eye(P, dtype=np.float16)
    in_maps = [
        {"fsrcT": fsT[b],
         "xyabc": np.ascontiguousarray(
             np.concatenate([xy9, abc9[b]], 1)),
         "ident": ident}
        for b in range(B)
    ]
    res = run_bass_kernel_spmd(_get_nc(), in_maps, list(range(B)), trace=TRACE)
    LAST_RESULTS = res
    outs = np.stack([res.results[b]["out"] for b in range(B)], 0)  # (B,N,C)
    return (outs.astype(np.float32) / np.float32(KAPPA)).reshape(B, C, H, W)
